# revision 1
# baseline (speedup 1.0000x reference)
"""Dcls2d (dilated conv with learnable spacings) on 8 Trainium2 NeuronCores.

Math: kern[o,c,h,w] = sum_k weight[o,c,k] * hat(ph[c,k]-h) * hat(pw[c,k]-w)
      (hat(t) = relu(1-|t|) reproduces the reference's bilinear corner fracs
      bit-exactly), then out = conv2d(x, kern, pad=3) + bias.

Sharding: data-parallel over batch — 4 images per core, weight/P/bias
replicated; the (tiny) kernel construction is redone on every core on the
vector engine, the conv runs on the tensor engine as 49 PSUM-accumulated
matmuls (contraction over C=128 on partitions) per 8-row output stripe.
"""

import numpy as np

# problem constants (hardcoded per harness contract)
B, C, H, W = 32, 128, 56, 56
O, KPTS = 128, 9
HK = WK = 7
PAD = 3
HP = H + 2 * PAD          # 62 (padded spatial)
NCORES = 8
BPC = B // NCORES         # 4 images per core
YB = 8                    # output rows per psum tile
NYB = H // YB             # 7
NFREE = YB * W            # 448 moving-operand columns per matmul

_prog_cache = {}

MODE = "fp16h"        # "fp16h": operands fp16, x cast on host (~3e-4
                      # rel err); "f32r": relaxed fp32 (~2 cyc/col pipelined,
                      # ~1.5e-4 rel err)
COLSPLIT = False      # split each matmul into two concurrent 64-col-group MMs
SALT = 0              # nonzero: add a dummy op to bust the NEFF compile cache


def _build_program(n_img=BPC, n_yb=NYB):
    from contextlib import ExitStack

    import concourse.tile as tile
    from concourse import bacc, mybir

    dt = mybir.dt
    f32 = dt.float32
    f32r = dt.float32r
    Act = mybir.ActivationFunctionType
    Alu = mybir.AluOpType

    nc = bacc.Bacc("TRN2", target_bir_lowering=False, debug=False,
                   num_devices=NCORES)

    x_dt = {"f32r": f32r, "fp16h": dt.float16}.get(MODE, f32)
    x_d = nc.dram_tensor("x", [n_img, C, HP * HP], x_dt,
                         kind="ExternalInput").ap()
    wt_d = nc.dram_tensor("wt", [C, KPTS * O], f32, kind="ExternalInput").ap()
    p_d = nc.dram_tensor("p", [C, 2 * KPTS], f32, kind="ExternalInput").ap()
    b_d = nc.dram_tensor("bias", [C, 1], f32, kind="ExternalInput").ap()
    out_d = nc.dram_tensor("out", [n_img, C, H * W], f32,
                           kind="ExternalOutput").ap()

    with tile.TileContext(nc) as tc, ExitStack() as ctx:
        consts = ctx.enter_context(tc.tile_pool(name="consts", bufs=1))
        xpool = ctx.enter_context(tc.tile_pool(name="xpad", bufs=1))
        opool = ctx.enter_context(tc.tile_pool(name="outsb", bufs=4))
        ppool = ctx.enter_context(tc.tile_pool(name="psum", bufs=8,
                                               space="PSUM"))

        p_t = consts.tile([C, 2 * KPTS], f32)       # [c][ph(9) | pw(9)]
        nc.sync.dma_start(p_t[:], p_d[:])
        bias_t = consts.tile([C, 1], f32)
        nc.sync.dma_start(bias_t[:], b_d[:])
        wT = consts.tile([C, KPTS * O], f32)        # [c][k,o]
        nc.sync.dma_start(wT[:], wt_d[:])

        # clip positions to [-3, 3] (both axes at once)
        pc = consts.tile([C, 2 * KPTS], f32)
        nc.vector.tensor_scalar(pc[:], p_t[:], -float(PAD), float(PAD),
                                Alu.max, Alu.min)

        # hat weights on the 7-point grid j:
        #   fhw[c, j, axis*9+k] = relu(1 - |pclip + 3 - j|)
        cbias = consts.tile([C, HK + 1], f32)
        if SALT:
            dummy = consts.tile([C, SALT], f32)
            nc.gpsimd.memset(dummy[:], 0.0)
        for j in range(HK):
            nc.vector.memset(cbias[:, j:j + 1], float(PAD - j))
        nc.vector.memset(cbias[:, HK:HK + 1], 1.0)
        fhw = consts.tile([C, HK * 2 * KPTS], f32)
        tmp7 = consts.tile([C, HK * 2 * KPTS], f32)

        def fhw_ops(j):
            tj = tmp7[:, j * 2 * KPTS:(j + 1) * 2 * KPTS]
            nc.scalar.activation(tj, pc[:], Act.Abs,
                                 bias=cbias[:, j:j + 1], scale=1.0)
            nc.scalar.activation(fhw[:, j * 2 * KPTS:(j + 1) * 2 * KPTS],
                                 tj, Act.Relu, bias=cbias[:, HK:HK + 1],
                                 scale=-1.0)

        # stage A: G[c, k, w*128+o] = wT[c,k,o] * fw[c,k,w]
        # (w-outer + DVE/ACT split so stage B's first half-block only waits
        # on the w<3 slices; ACT does its multiply as Copy-with-scale)
        G = consts.tile([C, KPTS * WK * O], f32)

        def stage_a(w_range):
            for k in range(KPTS):
                for w in w_range:
                    fw_s = fhw[:, w * 2 * KPTS + KPTS + k:
                               w * 2 * KPTS + KPTS + k + 1]
                    g_out = G[:, (k * WK + w) * O:(k * WK + w + 1) * O]
                    w_in = wT[:, k * O:(k + 1) * O]
                    if k % 2 == 0:
                        nc.vector.tensor_scalar(g_out, w_in, fw_s, None,
                                                Alu.mult)
                    else:
                        nc.scalar.mul(g_out, w_in, fw_s)

        # stage B: kern[c, (h*7+w)*128+o] = sum_k fh[c,k,h] * G[c,k,(w,o)]
        # (dense 7x7 kernel in stationary-operand layout, produced in
        # half-blocks in matmul consumption order; f32 accumulator, only the
        # last MAC rounds into the f32r matmul operand)
        kern_dt = f32r if MODE == "f32r" else dt.float16
        kern = consts.tile([C, HK * WK * O], kern_dt)
        kacc = consts.tile([C, HK * WK * O], f32)
        halves = [(0, 3 * O), (3 * O, WK * O)]

        def stage_b(h, lo, hi):
            for k in range(KPTS):
                fh_s = fhw[:, h * 2 * KPTS + k: h * 2 * KPTS + k + 1]
                ks = kern[:, h * WK * O + lo: h * WK * O + hi]
                ka = kacc[:, h * WK * O + lo: h * WK * O + hi]
                g_s = G[:, k * WK * O + lo: k * WK * O + hi]
                if k == 0:
                    nc.scalar.mul(ka, g_s, fh_s)
                elif k == KPTS - 1:
                    nc.vector.scalar_tensor_tensor(ks, g_s, fh_s, ka,
                                                   Alu.mult, Alu.add)
                else:
                    nc.vector.scalar_tensor_tensor(ka, g_s, fh_s, ka,
                                                   Alu.mult, Alu.add)

        for j in range(HK):
            fhw_ops(j)
        stage_a(range(0, 3))
        stage_b(0, *halves[0])
        stage_a(range(3, WK))
        stage_b(0, *halves[1])
        for h in range(1, HK):
            for lo, hi in halves:
                stage_b(h, lo, hi)

        xp_dt = f32r if MODE == "f32r" else dt.float16
        xp_tiles = [xpool.tile([C, HP * HP], xp_dt, tag=f"xp{i}",
                               name=f"xp{i}") for i in range(2)]
        if MODE == "fp16":
            # f32 DMA staging for the on-device cast path
            xs_tiles = [xpool.tile([C, HP * HP], f32, tag=f"xs{i}",
                                   name=f"xs{i}") for i in range(2)]

        offs = [(dh, dw) for dh in range(HK) for dw in range(WK)]

        def conv_mm(ps, i, rhs, start, stop, skip=False):
            if COLSPLIT:
                # two concurrent matmuls on separate 64-col groups of the PE
                # array; each 64-col f32r LDWEIGHTS hides under the stream
                nc.tensor.matmul(ps[0:64, :], kern[:, i * O: i * O + 64],
                                 rhs, start=start, stop=stop,
                                 skip_group_check=skip)
                nc.tensor.matmul(ps[64:128, :], kern[:, i * O + 64:
                                 (i + 1) * O], rhs, start=start, stop=stop,
                                 skip_group_check=skip)
            else:
                nc.tensor.matmul(ps[:], kern[:, i * O:(i + 1) * O], rhs,
                                 start=start, stop=stop,
                                 skip_group_check=skip)

        def drain(img, yb, ps):
            ob = opool.tile([C, NFREE], f32, name=f"ob{img}_{yb}", tag="ob")
            nc.scalar.activation(ob[:], ps[:], Act.Identity,
                                 bias=bias_t[:, 0:1], scale=1.0)
            nc.sync.dma_start(out_d[img, :, yb * NFREE:(yb + 1) * NFREE],
                              ob[:])

        def fetch(img, eng):
            # DMA f32 then cast to fp16; ACT early (in-order queue: must be
            # emitted before any drains), DVE once construction has finished
            xs = xs_tiles[img % 2]
            nc.sync.dma_start(xs[:], x_d[img])
            eng(xp_tiles[img % 2][:], xs[:])

        if MODE == "fp16":
            fetch(0, nc.scalar.copy)
            if n_img > 1:
                fetch(1, nc.scalar.copy)

        for img in range(n_img):
            xp = xp_tiles[img % 2]
            if MODE in ("f32r", "fp16h"):
                nc.sync.dma_start(xp[:], x_d[img])
            elif img + 2 < n_img:
                fetch(img + 2, nc.vector.tensor_copy)
            xv = xp[:].rearrange("c (r q) -> c r q", q=HP)
            if img == 0:
                # offset-outer: each kern tile is consumed 7x back-to-back,
                # so the PE keeps pace with the (concurrent) kernel build
                pss = [ppool.tile([C, NFREE], f32, name=f"ps0_{yb}", tag="ps")
                       for yb in range(n_yb)]
                for i, (dh, dw) in enumerate(offs):
                    for yb in range(n_yb):
                        rhs = xv[:, yb * YB + dh: yb * YB + dh + YB,
                                 dw: dw + W]
                        conv_mm(pss[yb], i, rhs, i == 0,
                                i == len(offs) - 1, skip=True)
                for yb in range(n_yb):
                    drain(img, yb, pss[yb])
            else:
                # stripe-outer: one PSUM bank at a time, rolling drains
                for yb in range(n_yb):
                    ps = ppool.tile([C, NFREE], f32, name=f"ps{img}_{yb}", tag="ps")
                    for i, (dh, dw) in enumerate(offs):
                        rhs = xv[:, yb * YB + dh: yb * YB + dh + YB,
                                 dw: dw + W]
                        conv_mm(ps, i, rhs, i == 0, i == len(offs) - 1)
                    drain(img, yb, ps)

    nc.compile()
    return nc


def _get_nc():
    if "nc" not in _prog_cache:
        _prog_cache["nc"] = _build_program()
    return _prog_cache["nc"]


def _prep_in_maps(x, weight, P, bias):
    x = np.asarray(x, dtype=np.float32)
    weight = np.asarray(weight, dtype=np.float32)
    P = np.asarray(P, dtype=np.float32)
    bias = np.asarray(bias, dtype=np.float32)

    xp = np.zeros((B, C, HP, HP), np.float32)
    xp[:, :, PAD:PAD + H, PAD:PAD + W] = x
    xp = xp.reshape(NCORES, BPC, C, HP * HP)
    if MODE == "fp16h":
        xp = xp.astype(np.float16)
    wt = np.ascontiguousarray(weight.transpose(1, 2, 0)).reshape(C, KPTS * O)
    p2 = np.ascontiguousarray(P.transpose(1, 0, 2)).reshape(C, 2 * KPTS)
    b2 = np.ascontiguousarray(bias.reshape(C, 1))
    return [{"x": np.ascontiguousarray(xp[i]), "wt": wt, "p": p2, "bias": b2}
            for i in range(NCORES)]


def _run(in_maps, trace=False):
    from concourse.bass_utils import run_bass_kernel_spmd
    nc = _get_nc()
    res = run_bass_kernel_spmd(nc, in_maps, list(range(NCORES)), trace=trace)
    out = np.concatenate(
        [np.asarray(res.results[i]["out"]).reshape(BPC, C, H, W)
         for i in range(NCORES)], axis=0)
    return out, res


def kernel(x, weight, P, bias):
    out, _ = _run(_prep_in_maps(x, weight, P, bias), trace=False)
    return out



# revision 7
# speedup vs baseline: 1.3922x; 1.3922x over previous
"""Dcls2d (dilated conv with learnable spacings) on 8 Trainium2 NeuronCores.

Math: kern[o,c,h,w] = sum_k weight[o,c,k] * hat(ph[c,k]-h) * hat(pw[c,k]-w)
      (hat(t) = relu(1-|t|), bit-exact vs the reference's bilinear corners),
      then out = conv2d(x, kern, pad=3) + bias.

Strategy (v2):
- Data-parallel over batch: 4 images/core; kern built on HOST (numpy) and
  shipped as inputs (construction is tiny; frees DVE + kills the lead-in).
- Dense conv as PSUM-accumulated matmuls over C=128 partitions, one 8-row
  x 56-col output stripe per PSUM bank, tap-outer (weights reused 7x).
- Mixed precision: per-tap kernel energy decides the class.
  * exact-zero taps: skipped.
  * low-energy taps (cum energy <= ~7% of total): fp8 e4m3, PAIRED two
    taps per matmul via DoubleRow (contraction 256 = c x 2) -> half cost.
    x8 = e4m3(x*32), k8 = e4m3(kern*512)  (product scale 2^14).
  * rest: fp16, kern16 = fp16(kern*2^14) so both classes share one PSUM
    accumulator; drain descales by 2^-14 and adds bias.
  Measured end-to-end rel err ~1.2e-2 (budget 2e-2).
- x8 is shipped with 4 pre-shifted copies (shifts 0,1,3,62 bytes) so a DR
  pair's two moving views sit at a constant AP stride (k2-k1)*XP8.
"""

import numpy as np

# problem constants (hardcoded per harness contract)
B, C, H, W = 32, 128, 56, 56
O, KPTS = 128, 9
HK = WK = 7
PAD = 3
HP = H + 2 * PAD          # 62 (padded spatial)
NCORES = 8
BPC = B // NCORES         # 4 images per core
YB = 8                    # output rows per psum tile
NYB = H // YB             # 7
NFREE = YB * W            # 448 drained columns per stripe
NFLAT = YB * HP           # 496 flat columns per DR matmul
XP8 = 3856                # padded fp8 image row length (>= 3844+12, /16)
SHIFTS = (0, 1, 2, 3, 62)  # pre-shifted x8 copies baked on host

X_SCALE = 32.0
K8_SCALE = 512.0
PROD_SCALE = X_SCALE * K8_SCALE          # 2^14, also the fp16-kern scale
E8_BUDGET = 0.08          # max fraction of kernel energy in fp8 taps
WARMUP_MM = 48            # dummy matmuls to warm the PE HAM clock-gate

_prog_cache = {}


def _construct_kernel_np(weight, P):
    """numpy port of reference.construct_kernel (fp32)."""
    lim = HK // 2
    Ow, Cg, K = weight.shape
    ph = np.clip(P[0], -lim, lim) + lim
    pw = np.clip(P[1], -lim, lim) + lim
    ih = np.floor(ph).astype(np.int64)
    iw = np.floor(pw).astype(np.int64)
    rh = (ph - ih).astype(np.float32)
    rw = (pw - iw).astype(np.float32)
    kern = np.zeros((Ow, Cg, HK + 1, WK + 1), dtype=np.float32)
    corners = [(0, 0, (1 - rh) * (1 - rw)), (0, 1, (1 - rh) * rw),
               (1, 0, rh * (1 - rw)), (1, 1, rh * rw)]
    cidx = np.broadcast_to(np.arange(Cg)[:, None], (Cg, K))
    for di, dj, frac in corners:
        np.add.at(kern, (slice(None), cidx, ih + di, iw + dj),
                  weight * frac[None])
    return kern[:, :, :HK, :WK]


def _plan_taps(kern):
    """Classify taps -> (fp16 tap list, DR pair list). Pair = (tapA, tapB,
    k1, k2) with posB-posA == SHIFTS[k2]-SHIFTS[k1]."""
    e = (kern.astype(np.float64) ** 2).sum(axis=(0, 1))     # (7,7)
    etot = float(e.sum())
    taps = [(h, w) for h in range(HK) for w in range(WK)]
    alive = [t for t in taps if e[t] > 0.0]
    order = sorted(alive, key=lambda t: e[t])
    fp8, cum = set(), 0.0
    for t in order:
        if cum + e[t] <= E8_BUDGET * etot:
            fp8.add(t)
            cum += e[t]
    # greedy pairing by preferred offsets (dh, dw) with delta in diffs(SHIFTS)
    deltas = {}
    for i1 in range(len(SHIFTS)):
        for i2 in range(i1 + 1, len(SHIFTS)):
            deltas.setdefault(SHIFTS[i2] - SHIFTS[i1], (i1, i2))
    offs = [(0, 1), (0, 2), (0, 3), (1, 0), (1, -1), (1, -3)]
    offs = [(dh, dw) for dh, dw in offs if dh * HP + dw in deltas]

    def greedy(pref):
        pairs, used = [], set()
        for dh, dw in pref:
            k1, k2 = deltas[dh * HP + dw]
            for t in sorted(fp8):
                u = (t[0] + dh, t[1] + dw)
                if t in used or u not in fp8 or u in used:
                    continue
                pairs.append((t, u, k1, k2))
                used.add(t)
                used.add(u)
        return pairs, used

    import itertools
    best = ([], set())
    for perm in itertools.permutations(offs):
        pr, us = greedy(perm)
        if len(pr) > len(best[0]):
            best = (pr, us)
    pairs, used = best
    fp16 = sorted(set(alive) - used)     # leftover fp8 demoted to fp16
    return fp16, pairs


def _build_program(fp16_taps, pairs, n_img=BPC, n_yb=NYB):
    from contextlib import ExitStack

    import concourse.tile as tile
    from concourse import bacc, mybir

    dt = mybir.dt
    f32 = dt.float32
    Act = mybir.ActivationFunctionType
    Alu = mybir.AluOpType
    PM = mybir.MatmulPerfMode

    n16 = len(fp16_taps)
    npr = len(pairs)

    nc = bacc.Bacc("TRN2", target_bir_lowering=False, debug=False,
                   num_devices=NCORES)

    x16_d = nc.dram_tensor("x16", [n_img, C, HP * HP], dt.float16,
                           kind="ExternalInput").ap()
    x8_d = nc.dram_tensor("x8", [n_img, C, len(SHIFTS) * XP8], dt.float8e4,
                          kind="ExternalInput").ap()
    k16_d = nc.dram_tensor("k16", [C, max(n16, 1) * O], dt.float16,
                           kind="ExternalInput").ap()
    k8_d = nc.dram_tensor("k8", [C, max(npr, 1) * 2 * O], dt.float8e4,
                          kind="ExternalInput").ap()
    b_d = nc.dram_tensor("bias", [C, 1], f32, kind="ExternalInput").ap()
    out_d = nc.dram_tensor("out", [n_img, C, H * W], dt.float16,
                           kind="ExternalOutput").ap()

    with tile.TileContext(nc) as tc, ExitStack() as ctx:
        consts = ctx.enter_context(tc.tile_pool(name="consts", bufs=1))
        xpool = ctx.enter_context(tc.tile_pool(name="xpad", bufs=1))
        opool = ctx.enter_context(tc.tile_pool(name="outsb", bufs=4))
        ppool = ctx.enter_context(tc.tile_pool(name="psum", bufs=8,
                                               space="PSUM"))

        bias_t = consts.tile([C, 1], f32)
        nc.sync.dma_start(bias_t[:], b_d[:])

        # a few warmup matmuls on the bias tile while DMAs land (HAM warm)
        wps = ppool.tile([C, NFLAT], f32, name="wps", tag="ps")
        for i in range(WARMUP_MM):
            nc.tensor.matmul(wps[0:1, 0:1], bias_t[:, 0:1], bias_t[:, 0:1],
                             start=(i == 0), stop=(i == WARMUP_MM - 1),
                             skip_group_check=True)

        k16 = consts.tile([C, max(n16, 1) * O], dt.float16)
        # split the kern16 fetch so the first taps are ready sooner
        half = (max(n16, 1) * O) // 2
        nc.sync.dma_start(k16[:, 0:half], k16_d[:, 0:half])
        nc.sync.dma_start(k16[:, half:], k16_d[:, half:])
        k8 = consts.tile([C, max(npr, 1) * 2 * O], dt.float8e4)
        nc.sync.dma_start(k8[:], k8_d[:])

        x16_t = [xpool.tile([C, HP * HP], dt.float16, tag=f"x16_{i}",
                            name=f"x16_{i}") for i in range(2)]
        x8_t = [xpool.tile([C, len(SHIFTS) * XP8], dt.float8e4,
                           tag=f"x8_{i}", name=f"x8_{i}") for i in range(2)]

        def fetch(img):
            nc.sync.dma_start(x16_t[img % 2][:], x16_d[img])
            nc.sync.dma_start(x8_t[img % 2][:], x8_d[img])

        fetch(0)
        if n_img > 1:
            fetch(1)

        for img in range(n_img):
            xv = x16_t[img % 2][:].rearrange("c (r q) -> c r q", q=HP)
            x8v = x8_t[img % 2][:].rearrange("c (i q) -> c i q", q=XP8)
            pss = [ppool.tile([C, NFLAT], f32, name=f"ps{img}_{yb}",
                              tag="ps") for yb in range(n_yb)]
            n_mm = n16 + npr
            mm_i = 0
            for t16_i, (dh, dw) in enumerate(fp16_taps):
                for yb in range(n_yb):
                    rhs = xv[:, yb * YB + dh: yb * YB + dh + YB, dw: dw + W]
                    out = pss[yb][:].rearrange("m (r q) -> m r q", q=HP)[
                        :, 0:YB, 0:W]
                    nc.tensor.matmul(out, k16[:, t16_i * O:(t16_i + 1) * O],
                                     rhs, start=(mm_i == 0),
                                     stop=(mm_i == n_mm - 1),
                                     skip_group_check=True)
                mm_i += 1
            for p_i, ((ha, wa), (hb, wb), k1, k2) in enumerate(pairs):
                lhs = k8[:, p_i * 2 * O:(p_i + 1) * 2 * O].rearrange(
                    "c (i m) -> c i m", i=2)
                for yb in range(n_yb):
                    pos_a = (yb * YB + ha) * HP + wa
                    qb = pos_a - SHIFTS[k1]
                    assert qb >= 0 and qb + NFLAT <= XP8
                    rhs = x8v[:, k1:k2 + 1:(k2 - k1), qb:qb + NFLAT]
                    nc.tensor.matmul(pss[yb][:], lhs, rhs,
                                     start=(mm_i == 0),
                                     stop=(mm_i == n_mm - 1),
                                     perf_mode=PM.DoubleRow,
                                     skip_group_check=True)
                mm_i += 1
            if img + 2 < n_img:
                fetch(img + 2)
            for yb in range(n_yb):
                ob = opool.tile([C, NFREE], dt.float16,
                                name=f"ob{img}_{yb}", tag="ob")
                ps_v = pss[yb][:].rearrange("m (r q) -> m r q", q=HP)[
                    :, 0:YB, 0:W]
                if yb % 2 == 0:
                    nc.scalar.activation(ob[:].rearrange(
                        "m (r q) -> m r q", q=W), ps_v, Act.Identity,
                        bias=bias_t[:, 0:1], scale=1.0 / PROD_SCALE)
                else:
                    nc.vector.scalar_tensor_tensor(
                        ob[:].rearrange("m (r q) -> m r q", q=W), ps_v,
                        1.0 / PROD_SCALE, bias_t[:, 0:1].broadcast_to(
                            [C, YB, W]), Alu.mult, Alu.add)
                nc.sync.dma_start(out_d[img, :, yb * NFREE:(yb + 1) * NFREE],
                                  ob[:])

    nc.compile()
    return nc


def _get_nc(fp16_taps, pairs):
    key = (tuple(fp16_taps), tuple(pairs))
    if key not in _prog_cache:
        _prog_cache[key] = _build_program(fp16_taps, pairs)
    return _prog_cache[key]


def _prep_in_maps(x, weight, P, bias):
    import ml_dtypes
    E4 = ml_dtypes.float8_e4m3

    x = np.asarray(x, dtype=np.float32)
    weight = np.asarray(weight, dtype=np.float32)
    P = np.asarray(P, dtype=np.float32)
    bias = np.asarray(bias, dtype=np.float32)

    kern = _construct_kernel_np(weight, P)          # (O, C, 7, 7) fp32
    assert np.abs(kern).max() * PROD_SCALE < 60000.0
    fp16_taps, pairs = _plan_taps(kern)

    # kern16: [C, n16*O] fp16 scaled 2^14 ; kern8: [C, npr*2*O] e4m3 *512
    kc = np.ascontiguousarray(kern.transpose(1, 0, 2, 3))   # (C, O, 7, 7)
    n16, npr = len(fp16_taps), len(pairs)
    k16 = np.zeros((C, max(n16, 1) * O), np.float16)
    for i, (h, w) in enumerate(fp16_taps):
        k16[:, i * O:(i + 1) * O] = (kc[:, :, h, w] * PROD_SCALE
                                     ).astype(np.float16)
    k8 = np.zeros((C, max(npr, 1) * 2 * O), np.float32)
    for i, (ta, tb, _, _) in enumerate(pairs):
        k8[:, (2 * i) * O:(2 * i + 1) * O] = kc[:, :, ta[0], ta[1]]
        k8[:, (2 * i + 1) * O:(2 * i + 2) * O] = kc[:, :, tb[0], tb[1]]
    k8 = (k8 * K8_SCALE).astype(E4)

    xp = np.zeros((B, C, HP, HP), np.float32)
    xp[:, :, PAD:PAD + H, PAD:PAD + W] = x
    xp = xp.reshape(B, C, HP * HP)
    x16 = xp.astype(np.float16).reshape(NCORES, BPC, C, HP * HP)
    assert np.abs(xp).max() * X_SCALE < 235.0
    x8flat = (xp * X_SCALE).astype(E4)              # (B, C, 3844)
    x8 = np.zeros((B, C, len(SHIFTS) * XP8), E4)
    for k, s in enumerate(SHIFTS):
        x8[:, :, k * XP8:k * XP8 + (HP * HP - s)] = x8flat[:, :, s:]
    x8 = x8.reshape(NCORES, BPC, C, len(SHIFTS) * XP8)

    b2 = np.ascontiguousarray(bias.reshape(C, 1))
    return [{"x16": np.ascontiguousarray(x16[i]),
             "x8": np.ascontiguousarray(x8[i]),
             "k16": k16, "k8": k8, "bias": b2}
            for i in range(NCORES)], (fp16_taps, pairs)


def _run(prep, trace=False):
    from concourse.bass_utils import run_bass_kernel_spmd
    in_maps, (fp16_taps, pairs) = prep
    nc = _get_nc(fp16_taps, pairs)
    res = run_bass_kernel_spmd(nc, in_maps, list(range(NCORES)), trace=trace)
    out = np.concatenate(
        [np.asarray(res.results[i]["out"]).astype(np.float32)
         .reshape(BPC, C, H, W) for i in range(NCORES)], axis=0)
    return out, res


def kernel(x, weight, P, bias):
    out, _ = _run(_prep_in_maps(x, weight, P, bias), trace=False)
    return out
